# revision 32
# baseline (speedup 1.0000x reference)
"""Trainium2 Bass kernel: PreActBlock with DoReFa 4-bit quantization (sync-BN).

  out = conv3x3(q(relu(BN1(conv3x3(q(relu(BN0(x))), qw(w0))))), qw(w1)) + x

Design (8 cores, data-parallel over batch 16 -> 2 images/core):
 - Quantized activations are integers 0..15 and quantized weights odd integers
   -15..15 (x scale).  Both are exact in fp8e4 (e4m3) and the PE accumulates
   in fp32, so every conv is computed EXACTLY as integer sums (|S| < 2^20).
 - fp8 DoubleRow matmuls: contraction K=256 per instruction via interleave.
 - Act layout [P, row, ki, 64]: the ki interleave dim sits between row and a
   64-wide padded row (ki step 64 satisfies DoubleRow's step%16==0), so each
   matmul's read range is a tight row window -> subtile deps let conv groups
   start as soon as the first quant blocks land.
 - conv0 integer outputs S stay in SBUF (bf16): no DRAM spill round-trip.
   BN1 stats from equal 392-elem bn_stats segments over S (bn_aggr requires
   equal counts).  x stays resident in SBUF for the residual add.
 - A dummy 1-element AllReduce issues first so the one-time comm-init
   rendezvous overlaps local work instead of gating the BN0 AllReduce.
 - BN coefficient math runs [P,2]-wide (both channel halves per op) with
   15*gamma / 15*beta precomputed off the critical path.
 - Dummy matmuls gated on each AllReduce result warm the PE clock (HAM)
   during the coeffs+quantize lead so conv runs at 2.4 GHz from the start.
"""
import os
import sys

sys.path.insert(0, "/opt/trn_rl_repo")

import numpy as np

import concourse.bacc as bacc
import concourse.bass as bass
import concourse.mybir as mybir
from concourse import tile
from concourse import bass_utils
from concourse.bass import bass_isa

F32 = mybir.dt.float32
BF16 = mybir.dt.bfloat16
FP8 = mybir.dt.float8e4
I8 = mybir.dt.int8
AX = mybir.AxisListType
OP = mybir.AluOpType
AF = mybir.ActivationFunctionType
PM = mybir.MatmulPerfMode
RO = bass_isa.ReduceOp

P = 128
N_CORES = 8
IMG = 2              # images per core
H = 56
CW = 64              # padded row width (ki step for DoubleRow must be %16==0)
ROWS = 116           # 2 images x (1 pad + 56 + 1 pad) rows
CNT = 50176.0        # global BN count: 16 * 56 * 56
EPS = 1e-5
N_WARM = (14, 10)    # HAM warm-up matmuls per conv (lead-time matched)

# 9-row output windows over logical rows 1..114
WINDOWS = [(1 + 9 * k, 9) for k in range(12)] + [(109, 6)]
# last group is a single window so the post-conv serial tail (stats flush /
# final stores) trails as little work as possible
GROUPS = [WINDOWS[0:2], WINDOWS[2:6], WINDOWS[6:10], WINDOWS[10:12],
          WINDOWS[12:13]]
# tap order: full-width tap (dy=0,dx=1) first so start=True covers all columns
TAPS = [(0, 1), (0, 0), (0, 2), (1, 0), (1, 1), (1, 2), (2, 0), (2, 1), (2, 2)]
# quantize blocks (img, h0, nrows): first two halved so conv can start early
QBLOCKS = [(0, 0, 14), (0, 14, 14), (0, 28, 28), (1, 0, 28), (1, 28, 28)]


def _runs(r0, nr):
    """Interior row-runs of a window: (logical_row, nrows, img, h0)."""
    out = []
    for lo, hi, img, base in ((1, 56, 0, 1), (59, 114, 1, 59)):
        a, b = max(r0, lo), min(r0 + nr - 1, hi)
        if a <= b:
            out.append((a, b - a + 1, img, a - base))
    return out


def build():
    nc = bacc.Bacc("TRN2", target_bir_lowering=False, debug=False,
                   enable_asserts=False, num_devices=N_CORES)

    x_d = nc.dram_tensor("x", [IMG, 256, H, H], F32, kind="ExternalInput")
    # host-permuted to [i, kh, kw, o]
    w_d = [nc.dram_tensor("conv0_w", [256, 3, 3, 256], F32, kind="ExternalInput"),
           nc.dram_tensor("conv1_w", [256, 3, 3, 256], F32, kind="ExternalInput")]
    g_d = [nc.dram_tensor("bn0_gamma", [256], F32, kind="ExternalInput"),
           nc.dram_tensor("bn1_gamma", [256], F32, kind="ExternalInput")]
    b_d = [nc.dram_tensor("bn0_beta", [256], F32, kind="ExternalInput"),
           nc.dram_tensor("bn1_beta", [256], F32, kind="ExternalInput")]
    out_d = nc.dram_tensor("out", [IMG, 256, H, H], F32, kind="ExternalOutput")

    xv = x_d.ap().rearrange("n c h w -> c n h w")       # [256, 2, 56, 56]
    ov = out_d.ap().rearrange("n c h w -> c n h w")

    with tile.TileContext(nc) as tc:
        with tc.tile_pool(name="act", bufs=1) as actp, \
             tc.tile_pool(name="wtp", bufs=1) as wtp, \
             tc.tile_pool(name="wq", bufs=4) as wqp, \
             tc.tile_pool(name="qt", bufs=3) as qtp, \
             tc.tile_pool(name="run", bufs=4) as runp, \
             tc.tile_pool(name="st", bufs=1) as stp, \
             tc.tile_pool(name="ps", bufs=8, space="PSUM") as psp, \
             tc.tile_pool(name="dram", bufs=1, space="DRAM") as drp:

            # ---------- static tiles ----------
            # activations: [P, row, ki, 64]; pad cols 56..63 are never read
            act0 = actp.tile([P, ROWS, 2, CW], FP8, name="act0")
            act1 = actp.tile([P, ROWS, 2, CW], FP8, name="act1")
            # quantized weight codes, [ci_lo, tap, ki, co] fp8
            wT = [wtp.tile([P, 9, 2, 256], FP8, name=f"w{v}T") for v in range(2)]
            # conv0 integer outputs, bf16, resident in SBUF
            Sst = [actp.tile([P, IMG, H, H], BF16, name=f"S_{c}")
                   for c in range(2)]
            # full x resident for stats0 / quant0 / residual
            xt = [actp.tile([P, IMG, H, H], F32, name=f"x_{c}")
                  for c in range(2)]
            warm = actp.tile([P, 2, 512], FP8, name="warm")
            ar_in = [drp.tile([P, 4], F32, name=f"ar_in_{i}") for i in range(3)]
            ar_out = [drp.tile([P, 4], F32, name=f"ar_out_{i}") for i in range(3)]

            # stats / small vectors; [P, 2] = both channel halves per column
            xbn = [stp.tile([P, 16, 6], F32, name=f"xbn_{c}") for c in range(2)]
            xbn1 = [stp.tile([P, 16, 6], F32, name=f"xbn1_{c}") for c in range(2)]
            gcat = [stp.tile([P, 2], F32, name=f"g_{v}") for v in range(2)]
            bcat = [stp.tile([P, 2], F32, name=f"b_{v}") for v in range(2)]
            g15 = [stp.tile([P, 2], F32, name=f"g15_{v}") for v in range(2)]
            b15 = [stp.tile([P, 2], F32, name=f"b15_{v}") for v in range(2)]
            avec = [stp.tile([P, 2], F32, name=f"a_{v}") for v in range(2)]
            bbvec = [stp.tile([P, 2], F32, name=f"bb_{v}") for v in range(2)]
            svec = [stp.tile([P, 1], F32, name=f"scale_{v}") for v in range(2)]
            s2t = stp.tile([P, 1], F32, name="s2")
            gs15 = stp.tile([P, 2], F32, name="gs15")
            # (sum, sumsq) x (c0, c1):  [:, c, 0]=sum  [:, c, 1]=sumsq
            pk = [stp.tile([P, 2, 2], F32, name=f"pk_{i}") for i in range(3)]
            gpk = [stp.tile([P, 2, 2], F32, name=f"gpk_{i}") for i in range(2)]
            gpkp = [stp.tile([P, 4], F32, name=f"gpkp_{i}") for i in range(2)]
            mvt = [stp.tile([P, 2, 2], F32, name=f"mv_{i}") for i in range(3)]

            def vtile(name, w=1):
                return stp.tile([P, w], F32, name=name, tag=f"vtmp{w}", bufs=8)

            # ---------- load BN params ----------
            for v in range(2):
                for c in range(2):
                    nc.gpsimd.dma_start(gcat[v][:, c:c + 1],
                                        g_d[v].ap()[c * P:(c + 1) * P])
                    nc.gpsimd.dma_start(bcat[v][:, c:c + 1],
                                        b_d[v].ap()[c * P:(c + 1) * P])
            for v in range(2):
                nc.vector.tensor_scalar(g15[v][:], gcat[v][:], 15.0, None,
                                        OP.mult)
                nc.vector.tensor_scalar(b15[v][:], bcat[v][:], 15.0, None,
                                        OP.mult)

            # ---------- x loads (sync queue) ----------
            for c in range(2):
                for img in range(IMG):
                    nc.sync.dma_start(xt[c][:, img],
                                      xv[c * P:(c + 1) * P, img])

            # ---------- act pad-row zeroing ----------
            with nc.named_scope("memset"):
                for t in (act0, act1):
                    for r in (0, 57, 58, 115):
                        nc.gpsimd.memset(t[:, r, :, :], 0.0)

            # local (sum, sumsq) over equal 392-elem bn_stats segments [sl]
            def seg_part(xb, sl, n, mv, pkt):
                for c in range(2):
                    nc.vector.bn_aggr(mv[:, c, :], xb[c][:, sl, :])
                nc.vector.tensor_scalar(pkt[:, :, 0], mv[:, :, 0],
                                        n, None, OP.mult)
                m2 = vtile("sm2", 2)
                nc.vector.tensor_mul(m2[:], mv[:, :, 0], mv[:, :, 0])
                vp = vtile("svp", 2)
                nc.vector.tensor_add(vp[:], mv[:, :, 1], m2[:])
                nc.vector.tensor_scalar(pkt[:, :, 1], vp[:],
                                        n, None, OP.mult)

            def seg_stats(src, xb, mv, pkt):
                for c in range(2):
                    fl = src(c)
                    for s in range(16):
                        nc.vector.bn_stats(xb[c][:, s, :],
                                           fl[:, s * 392:(s + 1) * 392])
                seg_part(xb, slice(0, 16), 6272.0, mv, pkt)

            # ---------- BN0 stats over x + AllReduce ----------
            with nc.named_scope("stats0"):
                seg_stats(lambda c: xt[c].rearrange("p i h w -> p (i h w)"),
                          xbn, mvt[0], pk[0])
                nc.sync.dma_start(ar_in[0][:],
                                  pk[0].rearrange("p a b -> p (a b)")[:])
                nc.gpsimd.collective_compute(
                    "AllReduce", OP.add, replica_groups=[list(range(N_CORES))],
                    ins=[ar_in[0].opt()], outs=[ar_out[0].opt()])

            # ---------- weight quantization ----------
            # DRAM layout [i, kh, kw, o] -> contiguous [ci_lo, tap, ki, co]
            def wquant(v):
                mxp = stp.tile([P, 4], F32, name=f"mxp_{v}")
                wv = w_d[v].ap().rearrange("i kh kw o -> i (kh kw) o")
                wnat = {}
                for ki in range(2):
                    for hh in range(2):  # tap halves: 0 -> taps 0..3, 1 -> 4..8
                        t0, t1 = (0, 4) if hh == 0 else (4, 9)
                        wn = wqp.tile([P, t1 - t0, 256], F32,
                                      name=f"wn{v}{ki}{hh}", tag="wnat", bufs=2,
                                      padded_shape=[P, 5, 256])
                        nc.scalar.dma_start(
                            wn[:], wv[ki * P:(ki + 1) * P, t0:t1, :])
                        wnat[(ki, hh)] = wn
                for i, (ki, hh) in enumerate(((0, 0), (0, 1), (1, 0), (1, 1))):
                    wn = wnat[(ki, hh)]
                    t = wqp.tile(list(wn.shape), F32, name=f"t{v}{ki}{hh}",
                                 tag="tanh", bufs=4, padded_shape=[P, 5, 256])
                    tf = t.rearrange("p a b -> p (a b)")
                    wf = wn.rearrange("p a b -> p (a b)")
                    nc.scalar.activation(tf[:], wf[:], AF.Tanh)
                    nc.vector.tensor_reduce(
                        mxp[:, i:i + 1], tf[:], AX.X, OP.max,
                        apply_absolute_value=True)
                    wnat[(ki, hh, "t")] = t
                mx1 = vtile(f"mx1_{v}")
                nc.vector.tensor_reduce(mx1[:], mxp[:], AX.X, OP.max,
                                        apply_absolute_value=True)
                mvec = vtile(f"mvec_{v}")
                nc.gpsimd.partition_all_reduce(mvec[:], mx1[:], P, RO.max)
                # svec = M/225 (psum scale); r = 7.5/M for codes
                nc.vector.tensor_scalar(svec[v][:], mvec[:], 1.0 / 225.0,
                                        None, OP.mult)
                r = vtile(f"rin_{v}")
                nc.vector.reciprocal(r[:], mvec[:])
                for i in range(2):  # Newton: r = r*(2 - M*r)
                    t1_ = vtile(f"rn1_{v}{i}")
                    nc.vector.tensor_mul(t1_[:], mvec[:], r[:])
                    t2_ = vtile(f"rn2_{v}{i}")
                    nc.vector.tensor_scalar(t2_[:], t1_[:], -1.0, 2.0,
                                            OP.mult, OP.add)
                    rn = vtile(f"rn3_{v}{i}")
                    nc.vector.tensor_mul(rn[:], r[:], t2_[:])
                    r = rn
                sc = vtile(f"sc_{v}")
                nc.vector.tensor_scalar(sc[:], r[:], 7.5, None, OP.mult)
                for i, (ki, hh) in enumerate(((0, 0), (0, 1), (1, 0), (1, 1))):
                    t = wnat[(ki, hh, "t")]
                    sh = list(t.shape)
                    tf = t.rearrange("p a b -> p (a b)")
                    z = wqp.tile(sh, F32, name=f"z{v}{ki}{hh}", tag="wz",
                                 bufs=2, padded_shape=[P, 5, 256])
                    zf = z.rearrange("p a b -> p (a b)")
                    nc.vector.tensor_scalar(zf[:], tf[:], sc[:], 7.5,
                                            OP.mult, OP.add)
                    ri = wqp.tile(sh, I8, name=f"ri{v}{ki}{hh}", tag="wr",
                                  bufs=2, padded_shape=[P, 5, 256])
                    rf = ri.rearrange("p a b -> p (a b)")
                    nc.vector.tensor_scalar(rf[:], zf[:], 0.0, 15.0,
                                            OP.max, OP.min)
                    t0 = 0 if hh == 0 else 4
                    dst = wT[v][:, t0:t0 + sh[1], ki, :]
                    nc.vector.tensor_scalar(dst, ri[:], 2.0, -15.0,
                                            OP.mult, OP.add)

            with nc.named_scope("wquant0"):
                wquant(0)
                # s2 and 15*g*svec0 for BN1 coeffs, off the critical path
                nc.vector.tensor_mul(s2t[:], svec[0][:], svec[0][:])
                nc.vector.tensor_scalar(gs15[:], g15[1][:], svec[0][:],
                                        None, OP.mult)
            with nc.named_scope("wquant1"):
                wquant(1)

            def gather_reduce(i):
                nc.sync.dma_start(gpk[i].rearrange("p a b -> p (a b)")[:],
                                  ar_out[i][:])

            # ---------- BN coeffs, [P,2]-wide: z = a*S + b ----------
            def bn_coeffs(v):
                gs = gpk[v][:, :, 0]
                gss = gpk[v][:, :, 1]
                mean = vtile(f"m{v}", 2)
                nc.vector.tensor_scalar(mean[:], gs, 1.0 / CNT, None, OP.mult)
                ex2 = vtile(f"e{v}", 2)
                nc.vector.tensor_scalar(ex2[:], gss, 1.0 / CNT, None, OP.mult)
                m2 = vtile(f"m2{v}", 2)
                nc.vector.tensor_mul(m2[:], mean[:], mean[:])
                d = vtile(f"d{v}", 2)
                nc.vector.tensor_sub(d[:], ex2[:], m2[:])
                vpe = vtile(f"vp{v}", 2)
                if v == 0:
                    nc.vector.tensor_scalar(vpe[:], d[:], EPS, None, OP.add)
                else:
                    nc.vector.tensor_scalar(vpe[:], d[:], s2t[:], EPS,
                                            OP.mult, OP.add)
                r = vtile(f"r{v}", 2)
                nc.vector.reciprocal(r[:], vpe[:])
                y = vtile(f"y{v}", 2)
                nc.scalar.activation(y[:], r[:], AF.Sqrt)
                # one Newton step: y *= 1.5 - 0.5*vpe*y^2
                y2 = vtile(f"y2{v}", 2)
                nc.vector.tensor_mul(y2[:], y[:], y[:])
                t2 = vtile(f"t2{v}", 2)
                nc.vector.tensor_mul(t2[:], vpe[:], y2[:])
                h = vtile(f"h{v}", 2)
                nc.vector.tensor_scalar(h[:], t2[:], -0.5, 1.5, OP.mult, OP.add)
                yn = vtile(f"yn{v}", 2)
                nc.vector.tensor_mul(yn[:], y[:], h[:])
                # a = 15*g*(scale)*rsqrt;  bb = 15*b - mean_S * a
                geff = g15[0] if v == 0 else gs15
                nc.vector.tensor_mul(avec[v][:], geff[:], yn[:])
                mg = vtile(f"mg{v}", 2)
                nc.vector.tensor_mul(mg[:], mean[:], avec[v][:])
                nc.vector.tensor_sub(bbvec[v][:], b15[v][:], mg[:])

            # The CoreSim scheduler does not model AllReduce latency, so it
            # would slot this gpk0-gated chain ahead of ready work in the
            # vector queue and head-of-line block it for the whole AR wait.
            # Push its virtual timestamp past all head-phase work instead.
            tc.tile_set_cur_wait(0.09)
            with nc.named_scope("coeffs0"):
                gather_reduce(0)
                bn_coeffs(0)

            # ---------- HAM warm-up matmuls, gated on the AR result ----------
            def warmup(v):
                nc.vector.tensor_scalar(warm[:, 0, 0:4],
                                        gpk[v].rearrange("p a b -> p (a b)")[:],
                                        0.0, None, OP.mult)
                wps = psp.tile([P, 9, H], F32, name=f"warm_ps{v}", tag="psw")
                lhsT = wT[v][:, 0, :, 0:P]
                for k in range(N_WARM[v]):
                    nc.tensor.matmul(wps[:, :, :], lhsT, warm[:, :, 0:504],
                                     start=True, stop=True,
                                     perf_mode=PM.DoubleRow,
                                     skip_group_check=True)

            warmup(0)

            # ---------- quantize: relu(a*in + b) -> min 15 -> int8 -> fp8 ----
            def quantize_layer(v, act, src_fn):
                for (img, h0, nr) in QBLOCKS:
                    lr = img * 58 + 1 + h0
                    for c in range(2):
                        nm = f"q{v}_{c}{img}{h0}"
                        z = qtp.tile([P, nr, H], F32, name=nm + "z",
                                     tag="qz", bufs=3, padded_shape=[P, 28, H])
                        nc.scalar.activation(z[:], src_fn(c, img, h0, nr),
                                             AF.Relu, bias=bbvec[v][:, c:c + 1],
                                             scale=avec[v][:, c:c + 1])
                        u = qtp.tile([P, nr, H], I8, name=nm + "u",
                                     tag="qu", bufs=3, padded_shape=[P, 28, H])
                        nc.vector.tensor_scalar(u[:], z[:], 15.0, None, OP.min)
                        nc.vector.tensor_copy(act[:, lr:lr + nr, c, 0:H], u[:])

            with nc.named_scope("quant0"):
                quantize_layer(
                    0, act0, lambda c, img, h0, nr: xt[c][:, img, h0:h0 + nr, :])
            tc.tile_set_cur_wait(0.0, enable=False)
            tc.cur_wait_ts = None

            # ---------- conv (shared), fp8 DoubleRow, K=256 per matmul ----------
            def conv(v, act, epilogue):
                for gi, grp in enumerate(GROUPS):
                    for co in range(2):
                        psums = []
                        for wi, (r0, nr) in enumerate(grp):
                            ps = psp.tile([P, nr, H], F32,
                                          name=f"ps{v}_{gi}_{co}_{wi}",
                                          tag="psw", padded_shape=[P, 9, H])
                            psums.append(ps)
                        for ti, (dy, dx) in enumerate(TAPS):
                            tap = dy * 3 + dx
                            wlo, whi = max(0, 1 - dx), min(H, H + 1 - dx)
                            jlo = max(0, dx - 1)
                            lhsT = wT[v][:, tap, :, co * P:(co + 1) * P]
                            first = ti == 0
                            last = ti == 8
                            for wi, (r0, nr) in enumerate(grp):
                                rows = slice(r0 + dy - 1, r0 + dy - 1 + nr)
                                rhs = act[:, rows, :, jlo:jlo + whi - wlo]
                                rhs = rhs.rearrange("p r k c -> p k r c")
                                out = psums[wi][:, :, wlo:whi]
                                nc.tensor.matmul(out, lhsT, rhs,
                                                 start=first, stop=last,
                                                 perf_mode=PM.DoubleRow)
                        for wi, (r0, nr) in enumerate(grp):
                            epilogue(co, r0, nr, psums[wi])

            # ---------- conv0 epilogue: copy S to SBUF (bf16) ----------
            def epi0(co, r0, nr, ps):
                for (rl, n, img, h0) in _runs(r0, nr):
                    nc.scalar.activation(Sst[co][:, img, h0:h0 + n, :],
                                         ps[:, rl - r0:rl - r0 + n, :],
                                         AF.Identity)

            with nc.named_scope("conv0"):
                conv(0, act0, epi0)

            # ---------- BN1 stats over S: split AllReduce ----------
            # Partial A (segments 0-11, img0 + img1 h<28) completes before the
            # last conv0 groups finish, so its AllReduce overlaps the conv0
            # tail; partial B (segments 12-15) rides a second, pipelined
            # collective.  AllReduce is linear, so summing the two global
            # partials reproduces the single-collective result exactly.
            with nc.named_scope("stats1"):
                for c in range(2):
                    fl = Sst[c].rearrange("p i h w -> p (i h w)")
                    for s in range(16):
                        nc.vector.bn_stats(xbn1[c][:, s, :],
                                           fl[:, s * 392:(s + 1) * 392])
                seg_part(xbn1, slice(0, 12), 4704.0, mvt[1], pk[1])
                nc.sync.dma_start(ar_in[1][:],
                                  pk[1].rearrange("p a b -> p (a b)")[:])
                nc.gpsimd.collective_compute(
                    "AllReduce", OP.add, replica_groups=[list(range(N_CORES))],
                    ins=[ar_in[1].opt()], outs=[ar_out[1].opt()])
                seg_part(xbn1, slice(12, 16), 1568.0, mvt[2], pk[2])
                nc.sync.dma_start(ar_in[2][:],
                                  pk[2].rearrange("p a b -> p (a b)")[:])
                nc.gpsimd.collective_compute(
                    "AllReduce", OP.add, replica_groups=[list(range(N_CORES))],
                    ins=[ar_in[2].opt()], outs=[ar_out[2].opt()])

            with nc.named_scope("coeffs1"):
                nc.sync.dma_start(gpkp[0][:], ar_out[1][:])
                nc.sync.dma_start(gpkp[1][:], ar_out[2][:])
                nc.vector.tensor_add(gpk[1].rearrange("p a b -> p (a b)")[:],
                                     gpkp[0][:], gpkp[1][:])
                bn_coeffs(1)

            warmup(1)

            # ---------- quantize1: S -> act1 codes ----------
            with nc.named_scope("quant1"):
                quantize_layer(
                    1, act1,
                    lambda c, img, h0, nr: Sst[c][:, img, h0:h0 + nr, :])

            # ---------- conv1 + residual epilogue ----------
            def epi1(co, r0, nr, ps):
                for (rl, n, img, h0) in _runs(r0, nr):
                    ot = runp.tile([P, n, H], F32, name=f"o_{co}_{rl}",
                                   tag="orun", bufs=4, padded_shape=[P, 9, H])
                    nc.vector.scalar_tensor_tensor(
                        ot[:], ps[:, rl - r0:rl - r0 + n, :], svec[1][:],
                        xt[co][:, img, h0:h0 + n, :], OP.mult, OP.add)
                    q = nc.sync if co == 0 else nc.scalar
                    q.dma_start(
                        ov[co * P:(co + 1) * P, img, h0:h0 + n, :], ot[:])

            with nc.named_scope("conv1"):
                conv(1, act1, epi1)

    nc.compile()
    return nc


def _install_ntff_hook():
    """Provide antenv.axon_hooks (absent in this image) via ctypes so that
    run_bass_kernel_spmd(trace=True) can capture NTFF profiles."""
    try:
        from antenv.axon_hooks import get_axon_ntff_profile_hook  # noqa: F401
        return
    except ImportError:
        pass
    import contextlib
    import ctypes
    import types

    so_path = "/opt/axon/libaxon_pjrt.so"
    if not os.path.exists(so_path):
        return
    lib = ctypes.CDLL(so_path)
    if not hasattr(lib, "axon_start_nrt_profile"):
        return
    lib.axon_start_nrt_profile.argtypes = [ctypes.POINTER(ctypes.c_int64),
                                           ctypes.c_size_t]
    lib.axon_start_nrt_profile.restype = ctypes.c_int64
    lib.axon_stop_nrt_profile.argtypes = [ctypes.c_char_p]
    lib.axon_stop_nrt_profile.restype = ctypes.c_int64

    @contextlib.contextmanager
    def _hook(output_dir, device_ids):
        import jax
        jax.devices()
        if device_ids:
            ids = (ctypes.c_int64 * len(device_ids))(*device_ids)
            rc = lib.axon_start_nrt_profile(ids, len(device_ids))
        else:
            rc = lib.axon_start_nrt_profile(None, 0)
        if rc != 0:
            raise RuntimeError(f"axon_start_nrt_profile rc={rc}")
        try:
            yield
        finally:
            n = lib.axon_stop_nrt_profile(str(output_dir).encode())
            print(f"ntff profile: {n} file(s) written to {output_dir}")

    hook_holder = [_hook]
    mod = types.ModuleType("antenv.axon_hooks")
    mod.get_axon_ntff_profile_hook = lambda: hook_holder[0]
    mod.set_axon_ntff_profile_hook = lambda h: hook_holder.__setitem__(0, h)
    import antenv
    sys.modules["antenv.axon_hooks"] = mod
    antenv.axon_hooks = mod


_NC = None


def _get_nc():
    global _NC
    if _NC is None:
        _NC = build()
    return _NC


LAST_RESULTS = None


def kernel(x, bn0_gamma, bn0_beta, conv0_w, bn1_gamma, bn1_beta, conv1_w):
    global LAST_RESULTS
    nc = _get_nc()
    shared = {
        # permute OIHW -> [i, kh, kw, o] so on-chip weight access is contiguous
        "conv0_w": np.ascontiguousarray(
            np.asarray(conv0_w, np.float32).transpose(1, 2, 3, 0)),
        "conv1_w": np.ascontiguousarray(
            np.asarray(conv1_w, np.float32).transpose(1, 2, 3, 0)),
        "bn0_gamma": np.ascontiguousarray(bn0_gamma, np.float32),
        "bn0_beta": np.ascontiguousarray(bn0_beta, np.float32),
        "bn1_gamma": np.ascontiguousarray(bn1_gamma, np.float32),
        "bn1_beta": np.ascontiguousarray(bn1_beta, np.float32),
    }
    x = np.ascontiguousarray(x, np.float32)
    in_maps = [{"x": x[2 * c:2 * c + 2], **shared} for c in range(N_CORES)]
    trace = bool(int(os.environ.get("KERNEL_TRACE", "0")))
    if trace:
        _install_ntff_hook()
    res = bass_utils.run_bass_kernel_spmd(
        nc, in_maps, core_ids=list(range(N_CORES)), trace=trace)
    LAST_RESULTS = res
    return np.concatenate([res.results[c]["out"] for c in range(N_CORES)], axis=0)
